# revision 1
# baseline (speedup 1.0000x reference)
"""DiffMamba kernel for 8 trn2 NeuronCores.

Sharding: head-parallel (16 DiffMamba heads x 2 teams = 32 Mamba2 modules;
core c owns heads {2c, 2c+1} for both teams). Each branch is computed with
the chunked-SSD formulation. The final concat + out-projection + residual
runs as a Bass SPMD kernel on the 8 cores (row-sharded over B*T); the
per-head Mamba branches feed it. A numpy path computes the branch math in
fp64 (exact vs the fp32 reference within float tolerance) and is also the
full fallback if the device path is unavailable.
"""
import os

import numpy as np

# ---- dims (hardcoded per problem spec) ----
D_MODEL = 256
HEADDIM = 16
NH = 16
D_STATE = 64
D_CONV = 4
D_INNER = 32
NH_IN = 2
CONV_DIM = 160
D_IN_PROJ = 194
B, T = 2, 2048
BT = B * T
N_CORES = 8
CHUNK = 128
L = 128             # SSD chunk length
NCH = T // L        # 16 chunks per batch element
NCH_RUN = NCH
KD_STAGE = 3
KD_SCAN = 4


def _softplus(v):
    return np.log1p(np.exp(-np.abs(v))) + np.maximum(v, 0.0)


def _silu(v):
    return v / (1.0 + np.exp(-v))


def _mamba2_batch(u, Win, convw, convb, dt_bias, A_log, Dp, nw, Wout):
    """Vectorized over leading module axis M. u: [M, B, T, HEADDIM] fp64.
    Chunked-SSD scan (algebraically identical to the sequential scan)."""
    M = u.shape[0]
    zxbcdt = np.einsum('mbtd,mpd->mbtp', u, Win)
    z = zxbcdt[..., :D_INNER]
    xBC = zxbcdt[..., D_INNER:D_INNER + CONV_DIM]
    dt = _softplus(zxbcdt[..., -NH_IN:] + dt_bias[:, None, None, :])  # [M,B,T,H]
    # causal depthwise conv over time
    xp = np.pad(xBC, ((0, 0), (0, 0), (D_CONV - 1, 0), (0, 0)))
    conv = convb[:, None, None, :] + sum(
        xp[:, :, k:k + T, :] * convw[:, None, None, :, k] for k in range(D_CONV))
    xBC = _silu(conv)
    x = xBC[..., :D_INNER].reshape(M, B, T, NH_IN, HEADDIM)
    Bm = xBC[..., D_INNER:D_INNER + D_STATE]
    Cm = xBC[..., D_INNER + D_STATE:]
    A = -np.exp(A_log)                       # [M, H]
    dtA = dt * A[:, None, None, :]           # [M, B, T, H]

    nch = T // CHUNK
    L = CHUNK
    dtA_c = dtA.reshape(M, B, nch, L, NH_IN)
    cum = np.cumsum(dtA_c, axis=3)                         # within-chunk cumsum
    x_c = x.reshape(M, B, nch, L, NH_IN, HEADDIM)
    B_c = Bm.reshape(M, B, nch, L, D_STATE)
    C_c = Cm.reshape(M, B, nch, L, D_STATE)
    dt_c = dt.reshape(M, B, nch, L, NH_IN)

    # intra-chunk: Y[t] = sum_{s<=t} C_t.B_s exp(cum_t - cum_s) dt_s x_s
    G = np.einsum('mbcln,mbcsn->mbcls', C_c, B_c)          # [M,B,c,L,L]
    seg = cum[:, :, :, :, None, :] - cum[:, :, :, None, :, :]  # [M,B,c,l,s,H]
    mask = np.tril(np.ones((L, L), dtype=bool))
    Emat = np.where(mask[None, None, None, :, :, None], np.exp(seg), 0.0)
    S = G[..., None] * Emat * dt_c[:, :, :, None, :, :]    # [M,B,c,l,s,H]
    Y = np.einsum('mbclsh,mbcshp->mbclhp', S, x_c)

    # chunk states: H_c = sum_s exp(cum_L - cum_s) dt_s x_s B_s^T
    w = np.exp(cum[:, :, :, -1:, :] - cum) * dt_c          # [M,B,c,L,H]
    Hc = np.einsum('mbclh,mbclhp,mbcln->mbchpn', w, x_c, B_c)
    # inter-chunk recurrence over nch chunks
    chunk_dec = np.exp(cum[:, :, :, -1, :])                # [M,B,c,H]
    states = np.zeros((M, B, NH_IN, HEADDIM, D_STATE))
    Yinter = np.empty_like(Y)
    for c in range(nch):
        Yinter[:, :, c] = np.einsum(
            'mbln,mblh,mbhpn->mblhp', C_c[:, :, c],
            np.exp(cum[:, :, c]), states)
        states = chunk_dec[:, :, c, :, None, None] * states + Hc[:, :, c]
    Y = Y + Yinter
    Y = Y + Dp[:, None, None, None, :, None] * x_c
    y = Y.reshape(M, B, T, D_INNER)
    y = y * _silu(z)
    y = y * (1.0 / np.sqrt(np.mean(y * y, axis=-1, keepdims=True) + 1e-5)) * nw[:, None, None, :]
    return np.einsum('mbtp,mdp->mbtd', y, Wout)


def _branches_numpy(x, params):
    """Compute normalized diff heads -> [B, T, D_MODEL] fp64 (pre out-proj)."""
    xh = np.moveaxis(x.reshape(B, T, NH, HEADDIM), 2, 0).astype(np.float64)
    (W1_in, W1_conv, W1_convb, W1_dtb, W1_Alog, W1_D, W1_nw, W1_out,
     W2_in, W2_conv, W2_convb, W2_dtb, W2_Alog, W2_D, W2_nw, W2_out,
     lam, gn_w, gn_b) = [np.asarray(p, np.float64) for p in params]
    y1 = _mamba2_batch(xh, W1_in, W1_conv, W1_convb, W1_dtb, W1_Alog, W1_D, W1_nw, W1_out)
    y2 = _mamba2_batch(xh, W2_in, W2_conv, W2_convb, W2_dtb, W2_Alog, W2_D, W2_nw, W2_out)
    diff = y1 - lam.reshape(NH, 1, 1, HEADDIM) * y2     # [NH,B,T,HD]
    mean = diff.mean(axis=(2, 3), keepdims=True)
    var = diff.var(axis=(2, 3), keepdims=True)
    diff = (diff - mean) / np.sqrt(var + 1e-5)
    diff = diff * gn_w[:, None, None, :] + gn_b[:, None, None, :]
    return np.moveaxis(diff, 0, 2).reshape(B, T, D_MODEL)


def _fix_bir_waits(js_bytes):
    """This walrus build allows 1 sync wait per instruction; split extras
    into single-wait EventSemaphore nops placed just before (same engine,
    same block => program order preserved per engine queue)."""
    import orjson
    js = orjson.loads(js_bytes)
    ctr = 0
    for fn in js.get("functions", []):
        for blk in fn.get("blocks", []):
            out = []
            for inst in blk.get("instructions", []):
                si = inst.get("sync_info")
                waits = (si or {}).get("on_wait") or []
                if len(waits) > 1:
                    for w in waits[:-1]:
                        out.append({
                            "debug": inst.get("debug", 0),
                            "engine": inst["engine"],
                            "ins": [], "outs": [],
                            "name": f"wsplit_{ctr}",
                            "opcode": "EventSemaphore",
                            "sync_info": {"on_update": [], "on_wait": [w]},
                        })
                        ctr += 1
                    si["on_wait"] = [waits[-1]]
                out.append(inst)
            blk["instructions"] = out
    return orjson.dumps(js)


def build_core_kernel():
    """Build the per-core Bass module (shared across all 8 cores)."""
    from concourse import bass, mybir
    import concourse.tile as tile

    f32 = mybir.dt.float32
    AL = mybir.AluOpType
    AF = mybir.ActivationFunctionType

    nc = bass.Bass()
    d = {}
    d['xt0'] = nc.declare_dram_parameter('xt0', [16, BT], f32, isOutput=False)
    d['xt1'] = nc.declare_dram_parameter('xt1', [16, BT], f32, isOutput=False)
    d['triu'] = nc.declare_dram_parameter('triu', [128, 128], f32, isOutput=False)
    d['ident'] = nc.declare_dram_parameter('ident', [128, 128], f32, isOutput=False)
    d['ones128'] = nc.declare_dram_parameter('ones128', [128, 128], f32, isOutput=False)
    d['ones16'] = nc.declare_dram_parameter('ones16', [1, 16], f32, isOutput=False)
    for m in range(4):
        d[f'winT{m}'] = nc.declare_dram_parameter(f'winT{m}', [16, 194], f32, isOutput=False)
        d[f'cwx{m}'] = nc.declare_dram_parameter(f'cwx{m}', [32, 4], f32, isOutput=False)
        d[f'cwb{m}'] = nc.declare_dram_parameter(f'cwb{m}', [64, 4], f32, isOutput=False)
        d[f'cwc{m}'] = nc.declare_dram_parameter(f'cwc{m}', [64, 4], f32, isOutput=False)
        d[f'cbx{m}'] = nc.declare_dram_parameter(f'cbx{m}', [32, 1], f32, isOutput=False)
        d[f'cbb{m}'] = nc.declare_dram_parameter(f'cbb{m}', [64, 1], f32, isOutput=False)
        d[f'cbc{m}'] = nc.declare_dram_parameter(f'cbc{m}', [64, 1], f32, isOutput=False)
        d[f'dtb{m}'] = nc.declare_dram_parameter(f'dtb{m}', [2, 1], f32, isOutput=False)
        d[f'A{m}'] = nc.declare_dram_parameter(f'A{m}', [2, 1], f32, isOutput=False)
        # lane-homed copies (DMA'd to the partition range where they are used)

        d[f'woutT{m}'] = nc.declare_dram_parameter(f'woutT{m}', [32, 16], f32, isOutput=False)
        d[f'Dp{m}'] = nc.declare_dram_parameter(f'Dp{m}', [128, 2], f32, isOutput=False)
    for hl in range(2):
        d[f'projT{hl}'] = nc.declare_dram_parameter(f'projT{hl}', [16, 256], f32, isOutput=False)
        d[f'gnw{hl}'] = nc.declare_dram_parameter(f'gnw{hl}', [16, 1], f32, isOutput=False)
        d[f'gnb{hl}'] = nc.declare_dram_parameter(f'gnb{hl}', [16, 1], f32, isOutput=False)
    out_ext = nc.declare_dram_parameter('out', [256, BT], f32, isOutput=True)

    with tile.TileContext(nc) as tc:
        with (
            tc.sbuf_pool(name='cp', bufs=1) as cp,      # consts/params, bufs=1
            tc.sbuf_pool(name='sp', bufs=2) as sp,      # per-module big tensors
            tc.sbuf_pool(name='wp', bufs=2) as wp,      # per-chunk work tiles
            tc.psum_pool(name='pin', bufs=2) as pin,    # in_proj + final proj psum
            tc.psum_pool(name='psc', bufs=2) as psc,    # scan psum
        ):
            # ---- load params ----
            sb = {}
            lane_home = {}
            for m in range(4):
                lane_home[f'cwx{m}'] = (32, 64)   # x lanes 32:64
                lane_home[f'cbx{m}'] = (32, 64)
                lane_home[f'dtb{m}'] = (64, 66)   # dt lanes 64:66
                lane_home[f'A{m}'] = (64, 66)
            for k, dr in d.items():
                shp = list(dr.shape)
                if k in lane_home:
                    lo, hi = lane_home[k]
                    tl = cp.tile([hi] + shp[1:], f32, tag=k)
                    nc.sync.dma_start(out=tl[lo:hi, :], in_=dr[:])
                    sb[k] = tl[lo:hi, :]
                else:
                    tl = cp.tile(shp, f32, tag=k)
                    nc.sync.dma_start(out=tl[:], in_=dr[:])
                    sb[k] = tl
            triu = sb['triu']; ident = sb['ident']
            ones128 = sb['ones128']; ones16 = sb['ones16']

            mods = {}  # per-module sbuf tensors

            def phaseA(m, b):
                """Lane-aligned layout (all elementwise ops stay on their lanes):
                tZX [66,T]: z 0:32 | x 32:64 | dtraw 64:66   (raw)
                tBr [64,T]: B raw 0:64
                tCd [64,T]: C raw 0:64
                tCB [64,T]: Bc (conv) 0:64
                tCC [64,T]: Cc (conv) 0:64
                tXc [66,T]: xc (conv) 32:64 | dtsp 64:66
                tDa [66,T]: dtA 64:66
                """
                hd = m // 2
                tZX = sp.tile([66, T], f32, tag='tZX', bufs=2)
                tBr = sp.tile([64, T], f32, tag='tBr', bufs=2)
                tCd = sp.tile([66, T], f32, tag='tCd', bufs=2)
                tCB = sp.tile([64, T], f32, tag='tCB', bufs=2)
                tCC = sp.tile([64, T], f32, tag='tCC', bufs=2)
                tXc = sp.tile([66, T], f32, tag='tXc', bufs=2)
                w = sb[f'winT{m}']
                rhsrows = sb[f'xt{hd}']
                cps = [nc.vector, nc.vector]

                def cpy(eng, dst, srcp):
                    if eng is nc.scalar:
                        nc.scalar.activation(dst, srcp, AF.Copy)
                    else:
                        eng.tensor_copy(dst, srcp)

                for t in range(8):
                    c0 = t * 256
                    g0 = b * T + c0
                    pzx = pin.tile([64, 256], f32, tag='pzx', bufs=1)
                    pB = pin.tile([64, 256], f32, tag='pB', bufs=1)
                    pCd = pin.tile([66, 256], f32, tag='pCd', bufs=1)
                    rh = rhsrows[:, g0:g0 + 256]
                    nc.tensor.matmul(pzx[:], w[:, 0:64], rh, start=True, stop=True)
                    nc.tensor.matmul(pB[:], w[:, 64:128], rh, start=True, stop=True)
                    nc.tensor.matmul(pCd[:], w[:, 128:194], rh, start=True, stop=True)
                    e0 = cps[t % 2]; e1 = cps[(t + 1) % 2]; e2 = cps[t % 2]
                    cpy(e0, tZX[0:64, c0:c0 + 256], pzx[:])          # z | x
                    cpy(e1, tBr[:, c0:c0 + 256], pB[:])              # B
                    cpy(e2, tCd[0:64, c0:c0 + 256], pCd[0:64, :])       # C
                    cpy(e1 if e1 is not nc.scalar else nc.vector,
                        tZX[64:66, c0:c0 + 256], pCd[64:66, :])      # dt raw
                # conv pieces (src, dst, lane range, weight, bias)
                for (srcap, dstap, lo, hi, wkey, bkey) in (
                    (tBr[0:64, :], tCB[0:64, :], 0, 64, f'cwb{m}', f'cbb{m}'),
                    (tCd[0:64, :], tCC[0:64, :], 0, 64, f'cwc{m}', f'cbc{m}'),
                    (tZX[32:64, :], tXc[32:64, :], 32, 64, f'cwx{m}', f'cbx{m}'),
                ):
                    cw = sb[wkey]; cb = sb[bkey]
                    P = hi - lo
                    nc.vector.tensor_scalar(dstap, srcap, cw[:, 3:4], cb[:],
                                            op0=AL.mult, op1=AL.add)
                    for k in range(3):
                        dsh = 3 - k
                        tmp = wp.tile([64, T], f32, tag='ctmp', bufs=1)
                        tm = tmp[lo:lo + P, 0:T - dsh]
                        if k % 2:
                            nc.gpsimd.tensor_scalar(tm, srcap[:, 0:T - dsh],
                                                    cw[:, k:k + 1], None, op0=AL.mult)
                        else:
                            nc.scalar.activation(tm, srcap[:, 0:T - dsh], AF.Copy,
                                                 scale=cw[:, k:k + 1])
                        nc.vector.tensor_tensor(dstap[:, dsh:T], dstap[:, dsh:T],
                                                tm, op=AL.add)
                    nc.scalar.activation(dstap, dstap, AF.Silu)
                # dt rows (lanes 64:66)
                nc.vector.tensor_scalar(tXc[64:66, :], tZX[64:66, :], sb[f'dtb{m}'],
                                        None, op0=AL.add)
                nc.scalar.activation(tXc[64:66, :], tXc[64:66, :], AF.Exp)
                nc.gpsimd.tensor_scalar(tXc[64:66, :], tXc[64:66, :], 1.0, None,
                                        op0=AL.add)
                nc.scalar.activation(tXc[64:66, :], tXc[64:66, :], AF.Ln)
                nc.vector.tensor_scalar(tCd[64:66, :], tXc[64:66, :], sb[f'A{m}'],
                                        None, op0=AL.mult)
                mods[m] = dict(tZX=tZX, tCB=tCB, tCC=tCC, tXc=tXc, tDa=tCd)

            _rot = {'pt': 0, 'pm': 0}

            def rot_tile(kind):
                i = _rot[kind] % 2
                _rot[kind] += 1
                return psc.tile([128, 512], f32, tag=f'{kind}{i}', bufs=1,
                                name=f'{kind}{i}_{_rot[kind]}')

            def scan_chunk(m, b, ci, S_tiles, team):
                """One (module, batch, chunk): returns ynT sbuf [32,128]."""
                md = mods[m]
                cs = slice(ci * L, ci * L + L)
                tA = md['tZX']; tAc = md['tXc']
                tZX = md['tZX']; tCB = md['tCB']; tCC = md['tCC']
                tXc = md['tXc']; tDa = md['tDa']
                Tsb = wp.tile([128, 134], f32, tag='Tsb', bufs=2)
                # transposes, each into its own rotating psum bank, copy out at once
                # Tsb cols: BT 0:64 | xT 64:96 | zT 96:128 | dtT 128:130 | dtAT 130:132
                for (dst_lo, dst_hi, srcap, idap) in (
                    (0, 64, tCB[0:64, cs], ident[0:64, 0:64]),
                    (64, 96, tXc[32:64, cs], ident[32:64, 32:64]),
                    (96, 128, tZX[0:32, cs], ident[0:32, 0:32]),
                    (128, 130, tXc[64:66, cs], ident[64:66, 64:66]),
                    (130, 132, tDa[64:66, cs], ident[64:66, 64:66]),
                ):
                    w = dst_hi - dst_lo
                    pt = rot_tile('pt')
                    nc.tensor.transpose(pt[:, 0:w], srcap, idap)
                    nc.vector.tensor_copy(Tsb[:, dst_lo:dst_hi], pt[:, 0:w])
                if KD_SCAN <= 1:
                    ynT = wp.tile([32, 128], f32, tag=f'ynT{team}', bufs=2)
                    nc.scalar.activation(ynT[:], Tsb[0:32, 0:128], AF.Copy)
                    return ynT
                # cumT[l,h] = sum_{s<=l} dtA_s
                pcum = rot_tile('pm')
                nc.tensor.matmul(pcum[:, 0:2], triu[:], Tsb[:, 130:132],
                                 start=True, stop=True)
                ctT = wp.tile([128, 2], f32, tag='ctT', bufs=2)
                nc.vector.tensor_copy(ctT[:], pcum[:, 0:2])
                # G'[s,l] = B_s . C_l ; causal mask
                pg = rot_tile('pm')
                nc.tensor.matmul(pg[:, 0:128], tCB[0:64, cs], tCC[0:64, cs],
                                 start=True, stop=True)
                Gm = wp.tile([128, 128], f32, tag='Gm', bufs=2)
                nc.vector.tensor_tensor(Gm[:], pg[:, 0:128], triu[:], op=AL.mult)
                # cum rows [1,128] for broadcast
                crs = []
                for h in range(2):
                    cr = wp.tile([1, 128], f32, tag=f'cr{h}', bufs=2)
                    pch = rot_tile('pt')
                    nc.tensor.transpose(pch[0:1, 0:128], ctT[:, h:h + 1], ident[:])
                    nc.vector.tensor_copy(cr[:], pch[0:1, 0:128])
                    crs.append(cr)
                if KD_SCAN <= 2:
                    ynT = wp.tile([32, 128], f32, tag=f'ynT{team}', bufs=2)
                    nc.scalar.activation(ynT[:], Gm[0:32, 0:128], AF.Copy)
                    return ynT
                ya = wp.tile([128, 32], f32, tag='ya', bufs=2)
                for h in range(2):
                    cumcol = ctT[:, h:h + 1]
                    dtcol = Tsb[:, 128 + h:129 + h]
                    xcols = Tsb[:, 64 + 16 * h:80 + 16 * h]
                    pbc = rot_tile('pm')
                    nc.tensor.matmul(pbc[:, 0:128], ones128[0:1, 0:128], crs[h][:],
                                     start=True, stop=True)
                    sarg = wp.tile([128, 128], f32, tag=f'sa{h}', bufs=2)
                    nc.vector.tensor_scalar(sarg[:], pbc[:, 0:128], cumcol, 0.0,
                                            op0=AL.subtract, op1=AL.min)
                    nc.scalar.activation(sarg[:], sarg[:], AF.Exp)
                    nc.vector.tensor_tensor(sarg[:], sarg[:], Gm[:], op=AL.mult)
                    wxdt = wp.tile([128, 16], f32, tag=f'wxdt{h}', bufs=2)
                    nc.scalar.activation(wxdt[:], xcols, AF.Copy, scale=dtcol)
                    py = rot_tile('pm')
                    nc.tensor.matmul(py[:, 0:16], sarg[:], wxdt[:],
                                     start=True, stop=True)
                    pyi = rot_tile('pm')
                    nc.tensor.matmul(pyi[:, 0:16], tCC[0:64, cs], S_tiles[h][:],
                                     start=True, stop=True)
                    # w path for chunk state
                    wcol = wp.tile([128, 1], f32, tag=f'wc{h}', bufs=2)
                    nc.vector.tensor_tensor(wcol[:], pbc[:, 127:128], cumcol,
                                            op=AL.subtract)
                    nc.scalar.activation(wcol[:], wcol[:], AF.Exp)
                    nc.vector.tensor_tensor(wcol[:], wcol[:], dtcol, op=AL.mult)
                    wx = wp.tile([128, 16], f32, tag=f'wx{h}', bufs=2)
                    nc.scalar.activation(wx[:], xcols, AF.Copy, scale=wcol[:])
                    sc = wp.tile([128, 1], f32, tag=f'sc{h}', bufs=2)
                    nc.scalar.activation(sc[:], cumcol, AF.Exp)
                    dc = wp.tile([128, 1], f32, tag=f'dc{h}', bufs=2)
                    nc.scalar.activation(dc[:], pbc[:, 127:128], AF.Exp)
                    phc = rot_tile('pm')
                    nc.tensor.matmul(phc[0:64, 0:16], Tsb[:, 0:64], wx[:],
                                     start=True, stop=True)
                    yint = wp.tile([128, 16], f32, tag=f'yi{h}', bufs=2)
                    nc.scalar.activation(yint[:], pyi[:, 0:16], AF.Copy, scale=sc[:])
                    yah = ya[:, 16 * h:16 * h + 16]
                    nc.vector.tensor_tensor(yah, py[:, 0:16], yint[:], op=AL.add)
                    yd = wp.tile([128, 16], f32, tag=f'yd{h}', bufs=2)
                    nc.scalar.activation(yd[:], xcols, AF.Copy,
                                         scale=sb[f'Dp{m}'][:, h:h + 1])
                    nc.vector.tensor_tensor(yah, yah, yd[:], op=AL.add)
                    S = S_tiles[h]
                    nc.vector.tensor_scalar(S[:], S[:], dc[0:64, 0:1], None, op0=AL.mult)
                    nc.vector.tensor_tensor(S[:], S[:], phc[0:64, 0:16], op=AL.add)
                if KD_SCAN <= 3:
                    ynT = wp.tile([32, 128], f32, tag=f'ynT{team}', bufs=2)
                    nc.scalar.activation(ynT[:], Gm[0:32, 0:128], AF.Copy)
                    return ynT
                # gate + rmsnorm (time-major [128, 32])
                sg = wp.tile([128, 32], f32, tag='sg', bufs=2)
                nc.scalar.activation(sg[:], Tsb[:, 96:128], AF.Silu)
                nc.vector.tensor_tensor(ya[:], ya[:], sg[:], op=AL.mult)
                scr = wp.tile([128, 32], f32, tag='scr', bufs=2)
                ms = wp.tile([128, 1], f32, tag='ms', bufs=2)
                nc.vector.tensor_tensor(scr[:], ya[:], ya[:], op=AL.mult)
                nc.vector.tensor_reduce(ms[:], scr[:], axis=mybir.AxisListType.X,
                                        op=AL.add)
                nc.vector.tensor_scalar(ms[:], ms[:], 1.0 / 32.0, 1e-5,
                                        op0=AL.mult, op1=AL.add)
                nc.scalar.activation(ms[:], ms[:], AF.Sqrt)
                rr = wp.tile([128, 1], f32, tag='rr', bufs=2)
                nc.vector.reciprocal(rr[:], ms[:])
                yn = wp.tile([128, 32], f32, tag='yn', bufs=2)
                nc.scalar.activation(yn[:], ya[:], AF.Copy, scale=rr[:])
                pyt = rot_tile('pt')
                nc.tensor.transpose(pyt[0:32, 0:128], yn[:], ident[:])
                ynT = wp.tile([32, 128], f32, tag=f'ynT{team}', bufs=2)
                nc.vector.tensor_copy(ynT[:], pyt[0:32, 0:128])
                return ynT

            def phaseB(hl):
                m0, m1 = 2 * hl, 2 * hl + 1
                diffT = sp.tile([16, BT], f32, tag='diffT', bufs=2)
                ssum = sp.tile([16, 32], f32, tag='ssum', bufs=2)
                ssq = sp.tile([16, 32], f32, tag='ssq', bufs=2)
                Sts = {}
                for m in (m0, m1):
                    for h in range(2):
                        S = sp.tile([64, 16], f32, tag=f'S{m % 2}_{h}', bufs=2)
                        Sts[(m, h)] = S
                for b in range(B):
                    phaseA(m0, b)
                    phaseA(m1, b)
                    for m in (m0, m1):
                        for h in range(2):
                            nc.gpsimd.memset(Sts[(m, h)][:], 0.0)
                    for ci in range(NCH_RUN):
                        yn0 = scan_chunk(m0, b, ci, [Sts[(m0, 0)], Sts[(m0, 1)]], 0)
                        yn1 = scan_chunk(m1, b, ci, [Sts[(m1, 0)], Sts[(m1, 1)]], 1)
                        pdt = rot_tile('pm')
                        pd = pdt[0:16, 0:128]
                        nc.tensor.matmul(pd[:], sb[f'woutT{m0}'][:], yn0[:],
                                         start=True, stop=False)
                        nc.tensor.matmul(pd[:], sb[f'woutT{m1}'][:], yn1[:],
                                         start=False, stop=True)
                        col = b * T + ci * L
                        nc.vector.tensor_copy(diffT[:, col:col + L], pd[:])
                        sc16 = wp.tile([16, 128], f32, tag='scr16', bufs=2)
                        nc.vector.tensor_reduce(ssum[:, b * 16 + ci:b * 16 + ci + 1],
                                                pd[:], axis=mybir.AxisListType.X,
                                                op=AL.add)
                        nc.vector.tensor_tensor(sc16[:], diffT[:, col:col + L],
                                                pd[:], op=AL.mult)
                        nc.vector.tensor_reduce(ssq[:, b * 16 + ci:b * 16 + ci + 1],
                                                sc16[:], axis=mybir.AxisListType.X,
                                                op=AL.add)
                return diffT, ssum, ssq

            def phaseC(hl, diffT, ssum, ssq):
                gnw = sb[f'gnw{hl}']; gnb = sb[f'gnb{hl}']
                for b in range(B):
                    r1 = wp.tile([16, 1], f32, tag='r1', bufs=2)
                    r2 = wp.tile([16, 1], f32, tag='r2', bufs=2)
                    nc.vector.tensor_reduce(r1[:], ssum[:, b * 16:b * 16 + 16],
                                            axis=mybir.AxisListType.X, op=AL.add)
                    nc.vector.tensor_reduce(r2[:], ssq[:, b * 16:b * 16 + 16],
                                            axis=mybir.AxisListType.X, op=AL.add)
                    c1 = wp.tile([1, 1], f32, tag='c1', bufs=2)
                    c2 = wp.tile([1, 1], f32, tag='c2', bufs=2)
                    nc.gpsimd.tensor_reduce(c1[:], r1[:], axis=mybir.AxisListType.C,
                                            op=AL.add)
                    nc.gpsimd.tensor_reduce(c2[:], r2[:], axis=mybir.AxisListType.C,
                                            op=AL.add)
                    NEL = 16.0 * T
                    m1 = wp.tile([1, 1], f32, tag='m1', bufs=2)
                    nc.vector.tensor_scalar(m1[:], c1[:], 1.0 / NEL, None, op0=AL.mult)
                    v1 = wp.tile([1, 1], f32, tag='v1', bufs=2)
                    nc.vector.tensor_scalar(v1[:], c2[:], 1.0 / NEL, None, op0=AL.mult)
                    msq = wp.tile([1, 1], f32, tag='msq', bufs=2)
                    nc.vector.tensor_tensor(msq[:], m1[:], m1[:], op=AL.mult)
                    nc.vector.tensor_tensor(v1[:], v1[:], msq[:], op=AL.subtract)
                    nc.vector.tensor_scalar(v1[:], v1[:], 1e-5, None, op0=AL.add)
                    nc.scalar.activation(v1[:], v1[:], AF.Sqrt)
                    rr = wp.tile([1, 1], f32, tag='rrg', bufs=2)
                    nc.vector.reciprocal(rr[:], v1[:])
                    pdt = rot_tile('pm')
                    pgn = pdt[0:16, 0:2]
                    nc.tensor.matmul(pgn[:, 0:1], ones16[:], m1[:], start=True, stop=True)
                    nc.tensor.matmul(pgn[:, 1:2], ones16[:], rr[:], start=True, stop=True)
                    scol = wp.tile([16, 1], f32, tag='scol', bufs=2)
                    nc.vector.tensor_tensor(scol[:], gnw[:], pgn[:, 1:2], op=AL.mult)
                    t1 = wp.tile([16, 1], f32, tag='t1g', bufs=2)
                    nc.vector.tensor_tensor(t1[:], pgn[:, 0:1], scol[:], op=AL.mult)
                    ocol = wp.tile([16, 1], f32, tag='ocol', bufs=2)
                    nc.vector.tensor_tensor(ocol[:], gnb[:], t1[:], op=AL.subtract)
                    nc.vector.tensor_scalar(diffT[:, b * T:(b + 1) * T],
                                            diffT[:, b * T:(b + 1) * T],
                                            scol[:], ocol[:], op0=AL.mult, op1=AL.add)

            # ---- run phases ----
            if KD_STAGE == 1:
                for b in range(B):
                    phaseA(0, b)
                    md = mods[0]
                    nc.sync.dma_start(out=out_ext[0:64, b * T:(b + 1) * T],
                                      in_=md['tCB'][0:64, :])
                    nc.sync.dma_start(out=out_ext[64:128, b * T:(b + 1) * T],
                                      in_=md['tCC'][0:64, :])
                    nc.sync.dma_start(out=out_ext[128:194, b * T:(b + 1) * T],
                                      in_=md['tXc'][0:66, :])
                    nc.sync.dma_start(out=out_ext[194:260 - 4, b * T:(b + 1) * T],
                                      in_=md['tZX'][0:62, :])
                raise_skip = True
            else:
                dT0, ss0, sq0 = phaseB(0)
                dT1, ss1, sq1 = phaseB(1)
            if KD_STAGE == 1:
                pass
            elif KD_STAGE == 2:
                nc.sync.dma_start(out=out_ext[0:16, :], in_=dT0[:])
                nc.sync.dma_start(out=out_ext[16:32, :], in_=dT1[:])
            else:
                phaseC(0, dT0, ss0, sq0)
                phaseC(1, dT1, ss1, sq1)
            # final projection: out[256, BT] partial = sum_h projT_h.T @ diffT_h
            for rg in range(2 if KD_STAGE == 3 else 0):
                for t in range(8):
                    c0 = t * 512
                    pp = pin.tile([128, 512], f32, tag='ps1', bufs=1)
                    nc.tensor.matmul(pp[:], sb['projT0'][:, rg * 128:rg * 128 + 128],
                                     dT0[:, c0:c0 + 512], start=True, stop=False)
                    nc.tensor.matmul(pp[:], sb['projT1'][:, rg * 128:rg * 128 + 128],
                                     dT1[:, c0:c0 + 512], start=False, stop=True)
                    ou = wp.tile([128, 512], f32, tag='outp', bufs=2)
                    eng = nc.vector
                    if eng is nc.scalar:
                        nc.scalar.activation(ou[:], pp[:], AF.Copy)
                    else:
                        eng.tensor_copy(ou[:], pp[:])
                    nc.sync.dma_start(out=out_ext[rg * 128:rg * 128 + 128, c0:c0 + 512],
                                      in_=ou[:])
    fixed = _fix_bir_waits(nc.to_json_bytes())
    nc.to_json_bytes = lambda: fixed
    return nc


def device_forward(x, params, trace=False):
    """Full DiffMamba forward on 8 cores. params = tuple as _branches_numpy
    plus (proj_w, proj_b) appended. Returns [B, T, D_MODEL] float32."""
    from concourse.bass_utils import run_bass_kernel_spmd

    (W1_in, W1_conv, W1_convb, W1_dtb, W1_Alog, W1_D, W1_nw, W1_out,
     W2_in, W2_conv, W2_convb, W2_dtb, W2_Alog, W2_D, W2_nw, W2_out,
     lam, gn_w, gn_b, proj_w, proj_b) = [np.asarray(p, np.float32) for p in params]
    x = np.asarray(x, np.float32)
    xT = np.ascontiguousarray(x.reshape(BT, D_MODEL).T)        # [256, BT]

    team = {
        1: (W1_in, W1_conv, W1_convb, W1_dtb, W1_Alog, W1_D, W1_nw, W1_out),
        2: (W2_in, W2_conv, W2_convb, W2_dtb, W2_Alog, W2_D, W2_nw, W2_out),
    }
    triu = np.triu(np.ones((128, 128), np.float32))            # [s<=l]
    ident = np.eye(128, dtype=np.float32)
    ones128 = np.ones((128, 128), np.float32)
    ones16 = np.ones((1, 16), np.float32)

    in_maps = []
    for c in range(N_CORES):
        H0 = 2 * c
        im = {'xt0': np.ascontiguousarray(xT[16 * H0:16 * H0 + 16]),
              'xt1': np.ascontiguousarray(xT[16 * H0 + 16:16 * H0 + 32]),
              'triu': triu, 'ident': ident, 'ones128': ones128, 'ones16': ones16}
        for m in range(4):
            H = H0 + m // 2
            tm = m % 2 + 1
            (Win, convw, convb, dtb, Alog, Dp, nw, Wout) = team[tm]
            im[f'winT{m}'] = np.ascontiguousarray(Win[H].T)           # [16,194]
            im[f'cwx{m}'] = np.ascontiguousarray(convw[H][0:32])      # [32,4]
            im[f'cwb{m}'] = np.ascontiguousarray(convw[H][32:96])     # [64,4]
            im[f'cwc{m}'] = np.ascontiguousarray(convw[H][96:160])    # [64,4]
            im[f'cbx{m}'] = np.ascontiguousarray(convb[H][0:32, None])
            im[f'cbb{m}'] = np.ascontiguousarray(convb[H][32:96, None])
            im[f'cbc{m}'] = np.ascontiguousarray(convb[H][96:160, None])
            im[f'dtb{m}'] = np.ascontiguousarray(dtb[H][:, None])
            im[f'A{m}'] = np.ascontiguousarray(-np.exp(Alog[H])[:, None])
            wt = (nw[H][:, None] * Wout[H].T).astype(np.float32)      # [32,16]
            if tm == 2:
                wt = -wt * lam[16 * H:16 * H + 16][None, :]
            im[f'woutT{m}'] = np.ascontiguousarray(wt)
            im[f'Dp{m}'] = np.ascontiguousarray(
                np.broadcast_to(Dp[H][None, :], (128, 2)).astype(np.float32))
        for hl in range(2):
            H = H0 + hl
            im[f'projT{hl}'] = np.ascontiguousarray(proj_w[:, 16 * H:16 * H + 16].T)
            im[f'gnw{hl}'] = np.ascontiguousarray(gn_w[H][:, None])
            im[f'gnb{hl}'] = np.ascontiguousarray(gn_b[H][:, None])
        in_maps.append(im)

    nc = build_core_kernel()
    res = run_bass_kernel_spmd(nc, in_maps, list(range(N_CORES)), trace=trace)
    total = np.zeros((256, BT), np.float64)
    for c in range(N_CORES):
        total += res.results[c]['out'].astype(np.float64)
    out = total.T.reshape(B, T, D_MODEL).astype(np.float32)
    return out + x + proj_b[None, None, :], res


_last_result = None


def kernel(x, W1_in, W1_conv, W1_convb, W1_dtb, W1_Alog, W1_D, W1_nw, W1_out,
           W2_in, W2_conv, W2_convb, W2_dtb, W2_Alog, W2_D, W2_nw, W2_out,
           lam, gn_w, gn_b, proj_w, proj_b):
    global _last_result
    params = [W1_in, W1_conv, W1_convb, W1_dtb, W1_Alog, W1_D, W1_nw, W1_out,
              W2_in, W2_conv, W2_convb, W2_dtb, W2_Alog, W2_D, W2_nw, W2_out,
              lam, gn_w, gn_b, proj_w, proj_b]
    trace = os.environ.get('KERNEL_TRACE', '') == '1'
    try:
        out, res = device_forward(x, params, trace=trace)
        _last_result = res
        return np.asarray(out, np.float32)
    except Exception:
        import traceback
        traceback.print_exc()
        diff = _branches_numpy(np.asarray(x), [np.asarray(p) for p in params[:-2]])
        out = (np.asarray(x, np.float64)
               + diff @ np.asarray(proj_w, np.float64).T
               + np.asarray(proj_b, np.float64))
        return out.astype(np.float32)

